# revision 1
# baseline (speedup 1.0000x reference)
import numpy as np
import jax
import jax.numpy as jnp
from jax.sharding import Mesh, NamedSharding, PartitionSpec as P

try:
    from jax.experimental.shard_map import shard_map
except ImportError:
    from jax import shard_map

# nn_GTN_58205396795517: 2-layer TransformerConv GNN
N = 100000
E = 800000
D = 64
H = 4
C = 64
M = 8

_INV_SQRT_C = np.float32(1.0 / np.sqrt(C))

_state = None


def _build():
    """Compile the pipeline as many small jits — the neuronx-cc backend
    crashes (DataLocalityOpt assert) on fused gather+arith graphs, so each
    stage is kept to a single primitive pattern known to compile."""
    global _state
    if _state is not None:
        return _state
    mesh = Mesh(np.array(jax.devices()[:M]), ('x',))
    rep = NamedSharding(mesh, P())
    esh = NamedSharding(mesh, P('x'))

    def smap(fn, in_specs, out_specs):
        return jax.jit(shard_map(fn, mesh=mesh, in_specs=in_specs,
                                 out_specs=out_specs))

    J = {}
    # dense projections (replicated on every core)
    J['dense'] = smap(lambda x, Wqkv, bqkv, Ws, bs:
                      tuple(jnp.split(x @ Wqkv + bqkv, 3, axis=1))
                      + (x @ Ws + bs,),
                      (P(),) * 5, (P(), P(), P(), P()))
    # row gathers (edge-sharded output)
    J['gather'] = smap(lambda t, i: t[i], (P(), P('x')), P('x'))
    # per-edge head-wise dot product
    J['dot'] = smap(lambda a, b:
                    (a * b).reshape(-1, H, C).sum(-1) * _INV_SQRT_C,
                    (P('x'), P('x')), P('x'))
    # exp (softmax without max-shift: logits are O(1), shift-invariant)
    J['exp'] = smap(lambda a: jnp.exp(a), (P('x'),), P('x'))
    # partial segment sum + all-reduce -> replicated node-indexed buffer
    J['segsum'] = smap(lambda v, i: jax.lax.psum(
        jax.ops.segment_sum(v, i, num_segments=N), 'x'),
        (P('x'), P('x')), P())
    # attn = ex / denom[dst]
    J['norm'] = smap(lambda ex, denom, i: ex / (denom[i] + 1e-16),
                     (P('x'), P(), P('x')), P('x'))
    # msg = v[src] * attn (attn broadcast across C within each head)
    J['msg'] = smap(lambda vs, at: vs * jnp.repeat(at, C, axis=1),
                    (P('x'), P('x')), P('x'))
    # head mean + skip connection
    J['out'] = smap(lambda agg, skip:
                    agg.reshape(N, H, C).mean(axis=1) + skip,
                    (P(), P()), P())
    J['relu'] = smap(lambda h: jax.nn.relu(h), (P(),), P())

    _state = (mesh, rep, esh, J)
    return _state


def _layer(x_d, src, dst, Wqkv, bqkv, Ws, bs, J):
    q, k, v, skip = J['dense'](x_d, Wqkv, bqkv, Ws, bs)
    qd = J['gather'](q, dst)
    ks = J['gather'](k, src)
    alpha = J['dot'](qd, ks)
    ex = J['exp'](alpha)
    denom = J['segsum'](ex, dst)
    attn = J['norm'](ex, denom, dst)
    vs = J['gather'](v, src)
    msg = J['msg'](vs, attn)
    agg = J['segsum'](msg, dst)
    return J['out'](agg, skip)


def kernel(x, edge_index, Wq1, bq1, Wk1, bk1, Wv1, bv1, Ws1, bs1,
           Wq2, bq2, Wk2, bk2, Wv2, bv2, Ws2, bs2):
    mesh, rep, esh, J = _build()

    ei = np.asarray(edge_index)
    src = jax.device_put(jnp.asarray(ei[0]), esh)
    dst = jax.device_put(jnp.asarray(ei[1]), esh)

    def prep(Wq, Wk, Wv, bq, bk, bv):
        Wqkv = np.concatenate([np.asarray(Wq), np.asarray(Wk),
                               np.asarray(Wv)], axis=1)
        bqkv = np.concatenate([np.asarray(bq), np.asarray(bk),
                               np.asarray(bv)])
        return (jax.device_put(jnp.asarray(Wqkv), rep),
                jax.device_put(jnp.asarray(bqkv), rep))

    W1, b1 = prep(Wq1, Wk1, Wv1, bq1, bk1, bv1)
    W2, b2 = prep(Wq2, Wk2, Wv2, bq2, bk2, bv2)
    pr = lambda a: jax.device_put(jnp.asarray(np.asarray(a)), rep)
    Ws1d, bs1d, Ws2d, bs2d = pr(Ws1), pr(bs1), pr(Ws2), pr(bs2)

    x_d = pr(x)
    h = _layer(x_d, src, dst, W1, b1, Ws1d, bs1d, J)
    h = J['relu'](h)
    out = _layer(h, src, dst, W2, b2, Ws2d, bs2d, J)
    return np.asarray(jax.device_get(out)).astype(np.float32)

